# revision 28
# baseline (speedup 1.0000x reference)
"""CapsAlexNet (FLOWER102) forward pass on 8 Trainium2 NeuronCores.

Sharding:
  - conv stack: data-parallel over batch (2 images/core); conv1 via host im2col.
  - capsule routing: capsule dim (i) sharded 8 ways (AllToAll switches from
    batch-sharding to i-sharding); x_hat computed once in pass 0 (bf16
    block-diagonal matmuls) and cached in SBUF; AllReduce of [16,102,16] per
    iteration.
  - FC head: output-sharded (512 cols/core) with AllGather between layers.
All matmuls run in bf16 (fp32 matmul is 4 cycles/row on TRN2, bf16 is 1).
"""

import numpy as np
from numpy.lib.stride_tricks import as_strided

import concourse.bass as bass
import concourse.mybir as mybir
import concourse.tile as tile
from concourse import bacc
from concourse.ap import AP
from concourse.bass_utils import run_bass_kernel_spmd

F32 = mybir.dt.float32
BF16 = mybir.dt.bfloat16
AX = mybir.AxisListType
OP = mybir.AluOpType
AF = mybir.ActivationFunctionType

NCORES = 8
B = 16
BC = 2           # images per core
O = 102
D = 16
OD = O * D       # 1632
ITOT = 2592
IPAD = 2688
ILOC = IPAD // NCORES   # 336
G = ILOC // 8           # 42
RG = [list(range(NCORES))]

_CACHE = {}


def _chunks512(total):
    """[0,total) split at 512-float PSUM bank boundaries."""
    out, s = [], 0
    while s < total:
        e = min(s + 512, total)
        out.append((s, e))
        s = e
    return out


def _dap(a, offset, dims):
    """Manual AP into a dram-pool tile (which is itself an AP)."""
    return AP(tensor=a.tensor, offset=a.offset + offset,
              ap=[list(d) for d in dims])


def _pv(t, part0, free0, dims):
    """AP into SBUF tile t at (partition part0, free offset free0).

    dims: list of [step, count] for the free dims; partition count inferred
    stays full unless npart given via dims[0] being ('P', count).
    """
    base = t[:]
    fs = base.ap[0][0]          # partition stride == free size
    npart = dims[0][1] if dims[0][0] == "P" else base.ap[0][1]
    rest = dims[1:] if dims[0][0] == "P" else dims
    return AP(tensor=base.tensor, offset=base.offset + part0 * fs + free0,
              ap=[[fs, npart]] + [list(d) for d in rest])


def build_program():
    nc = bacc.Bacc("TRN2", target_bir_lowering=False, debug=False,
                   num_devices=NCORES)

    def din(name, shape, dt=F32):
        return nc.declare_dram_parameter(name, list(shape), dt, isOutput=False)

    xcols = din("xcols", [BC, 363, 2601], BF16)
    w1T = din("w1T", [128, 288], BF16); b1c = din("b1c", [96, 1])
    w2T = din("w2T", [96, 6400], BF16); b2c = din("b2c", [2, 128, 1])
    wpT = din("wpT", [128, 8192], BF16); bpc = din("bpc", [2, 128, 1])
    w3T = din("w3T", [3, 768], BF16); b3c = din("b3c", [2, 128, 1])
    wrg = din("wrg", [G, 64, OD], BF16)
    smat = din("smat", [128, 16], BF16)
    fw1T = din("fw1T", [36, 128, 2048], BF16)
    fb1r = din("fb1r", [16, 512])
    fw2T = din("fw2T", [8, 128, 2048], BF16)
    fb2r = din("fb2r", [16, 512])
    fw3T = din("fw3T", [8, 128, 408], BF16)
    fb3r = din("fb3r", [16, 102])
    out_t = nc.declare_dram_parameter("out", [B, O], F32, isOutput=True)

    with tile.TileContext(nc) as tc:
        with tc.tile_pool(name="dram", bufs=1, space="DRAM") as dram:
            _build_body(nc, tc, dram, locals())
    nc.finalize()
    return nc


def _build_body(nc, tc, dram, T):
    xcols, w1T, b1c, w2T, b2c, wpT, bpc = (T["xcols"], T["w1T"], T["b1c"],
                                           T["w2T"], T["b2c"], T["wpT"], T["bpc"])
    w3T, b3c, wrg, smat = T["w3T"], T["b3c"], T["wrg"], T["smat"]
    fw1T, fb1r, fw2T, fb2r, fw3T, fb3r = (T["fw1T"], T["fb1r"], T["fw2T"],
                                          T["fb2r"], T["fw3T"], T["fb3r"])
    out_t = T["out_t"]

    # ---------------- DRAM scratch ----------------
    upc = dram.tile([BC, 20736], F32, tag="upc")
    u_loc = dram.tile([BC, IPAD * 8], BF16, tag="uloc")        # [2, 21504]
    u_a2a = dram.tile([NCORES, BC, ILOC * 8], BF16, tag="ua2a")  # [8,2,2688]
    u_mine = dram.tile([NCORES, BC, ILOC * 8], BF16, tag="umine")
    v_in = [dram.tile([16, OD], BF16, tag=f"vin{i}", name=f"vin{i}")
            for i in range(3)]
    v_out = [dram.tile([16, OD], BF16, tag=f"vout{i}", name=f"vout{i}")
             for i in range(3)]
    v2d = dram.tile([B * OD], BF16, tag="v2d")
    fD2 = dram.tile([18432, B], BF16, tag="fD2")
    f1loc = dram.tile([512, B], F32, tag="f1loc")
    f1g = dram.tile([4096, B], F32, tag="f1g")
    f2loc = dram.tile([512, B], F32, tag="f2loc")
    f2g = dram.tile([4096, B], F32, tag="f2g")

    # =========================================================
    # Phase A: conv stack
    # =========================================================
    with (
        tc.tile_pool(name="caw", bufs=1) as cw,
        tc.tile_pool(name="cact", bufs=1) as ca,
        tc.tile_pool(name="cps", bufs=2, space="PSUM") as cps,
        tc.tile_pool(name="cps1", bufs=1, space="PSUM") as cps1,
    ):
        w1t_sb = cw.tile([128, 3 * 96], BF16, tag="w1t")
        nc.sync.dma_start(out=w1t_sb[:], in_=w1T[:, :])
        w2t_sb = cw.tile([96, 25 * 256], BF16, tag="w2t")
        nc.gpsimd.dma_start(out=w2t_sb[:], in_=w2T[:, :])
        wpt_sb = cw.tile([128, 32 * 256], BF16, tag="wpt")
        nc.gpsimd.dma_start(out=wpt_sb[:], in_=wpT[:, :])
        b1_sb = cw.tile([96, 1], F32, tag="b1s")
        nc.sync.dma_start(out=b1_sb[:], in_=b1c[:, :])
        b2_sb = cw.tile([128, 2], F32, tag="b2s")
        nc.sync.dma_start(out=b2_sb[:].rearrange("c (m one) -> c m one", one=1),
                          in_=b2c.ap().rearrange("m c one -> c m one"))
        bp_sb = cw.tile([128, 2], F32, tag="bps")
        nc.sync.dma_start(out=bp_sb[:].rearrange("c (m one) -> c m one", one=1),
                          in_=bpc.ap().rearrange("m c one -> c m one"))

        # ---- conv1 + relu ----
        xc_sb = ca.tile([128, BC * 3 * 2601], BF16, tag="xc")
        for img in range(BC):
            for kt in range(3):
                rows = 128 if kt < 2 else 107
                c0 = (img * 3 + kt) * 2601
                nc.sync.dma_start(out=xc_sb[:rows, c0:c0 + 2601],
                                  in_=xcols[img, kt * 128:kt * 128 + rows, :])
        h1_sb = ca.tile([96, BC * 2601], BF16, tag="h1")
        for img in range(BC):
            for (n0, n1) in _chunks512(2601):
                ps = cps.tile([96, 512], F32, tag="ps1")
                for kt in range(3):
                    rows = 128 if kt < 2 else 107
                    c0 = (img * 3 + kt) * 2601
                    nc.tensor.matmul(ps[:, :n1 - n0],
                                     w1t_sb[:rows, kt * 96:(kt + 1) * 96],
                                     xc_sb[:rows, c0 + n0:c0 + n1],
                                     start=(kt == 0), stop=(kt == 2))
                nc.scalar.activation(h1_sb[:, img * 2601 + n0:img * 2601 + n1],
                                     ps[:, :n1 - n0], AF.Relu, bias=b1_sb[:, 0:1])

        # ---- maxpool1 -> write into padded conv2 input ----
        p1p_sb = ca.tile([96, BC * 841], BF16, tag="p1p")
        nc.vector.memset(p1p_sb[:], 0.0)
        for img in range(BC):
            def h1v(ky, kx):
                return _pv(h1_sb, 0, img * 2601 + ky * 51 + kx,
                           [[102, 25], [2, 25]])
            acc = ca.tile([96, 625], BF16, tag="pool1")
            a3 = acc[:].rearrange("p (a b) -> p a b", b=25)
            nc.vector.tensor_max(a3, h1v(0, 0), h1v(0, 1))
            for t in range(2, 9):
                ky, kx = divmod(t, 3)
                nc.vector.tensor_max(a3, a3, h1v(ky, kx))
            dst = _pv(p1p_sb, 0, img * 841 + 2 * 29 + 2, [[29, 25], [1, 25]])
            nc.vector.tensor_copy(out=dst, in_=a3)

        # ---- conv2 + relu ----
        h2_sb = ca.tile([128, 2 * BC * 625], BF16, tag="h2")
        for mch in range(2):
            ps2 = {}
            for img in range(BC):
                for nch in range(2):
                    ps2[(img, nch)] = cps1.tile(
                        [128, 512], F32, tag=f"ps2_{img}_{nch}",
                        name=f"ps2_{img}_{nch}")
            for tap in range(25):
                ky, kx = divmod(tap, 5)
                lhs = w2t_sb[:, tap * 256 + mch * 128:tap * 256 + (mch + 1) * 128]
                for img in range(BC):
                    for nch, (oy0, nyy) in enumerate([(0, 13), (13, 12)]):
                        rhs = _pv(p1p_sb, 0,
                                  img * 841 + (oy0 + ky) * 29 + kx,
                                  [[29, nyy], [1, 25]])
                        nc.tensor.matmul(ps2[(img, nch)][:, :nyy * 25], lhs, rhs,
                                         start=(tap == 0), stop=(tap == 24))
            for img in range(BC):
                for nch, (oy0, nyy) in enumerate([(0, 13), (13, 12)]):
                    nc.scalar.activation(
                        h2_sb[:, (mch * BC + img) * 625 + oy0 * 25:
                              (mch * BC + img) * 625 + (oy0 + nyy) * 25],
                        ps2[(img, nch)][:, :nyy * 25], AF.Relu,
                        bias=b2_sb[:, mch:mch + 1])

        # ---- maxpool2 ----
        p2_sb = ca.tile([128, 2 * BC * 144], BF16, tag="p2")
        for mch in range(2):
            for img in range(BC):
                base = (mch * BC + img) * 625
                def h2v(ky, kx):
                    return _pv(h2_sb, 0, base + ky * 25 + kx,
                               [[50, 12], [2, 12]])
                dst = p2_sb[:, (mch * BC + img) * 144:(mch * BC + img + 1) * 144]
                d3 = dst.rearrange("p (a b) -> p a b", b=12)
                nc.vector.tensor_max(d3, h2v(0, 0), h2v(0, 1))
                for t in range(2, 9):
                    ky, kx = divmod(t, 3)
                    nc.vector.tensor_max(d3, d3, h2v(ky, kx))

        # ---- primarycaps conv (no relu) ----
        pc_sb = ca.tile([128, 2 * BC * 81], F32, tag="pc")
        for mch in range(2):
            psP = {img: cps1.tile([128, 81], F32, tag=f"psP_{img}",
                                  name=f"psP_{img}")
                   for img in range(BC)}
            for tap in range(16):
                ky, kx = divmod(tap, 4)
                for kc in range(2):
                    lhs = wpt_sb[:, (tap * 2 + kc) * 256 + mch * 128:
                                 (tap * 2 + kc) * 256 + (mch + 1) * 128]
                    for img in range(BC):
                        rhs = _pv(p2_sb, 0, (kc * BC + img) * 144 + ky * 12 + kx,
                                  [[12, 9], [1, 9]])
                        nc.tensor.matmul(psP[img][:], lhs, rhs,
                                         start=(tap == 0 and kc == 0),
                                         stop=(tap == 15 and kc == 1))
            for img in range(BC):
                nc.vector.tensor_scalar_add(
                    pc_sb[:, (mch * BC + img) * 81:(mch * BC + img + 1) * 81],
                    psP[img][:], bp_sb[:, mch:mch + 1])

        for mch in range(2):
            for img in range(BC):
                nc.sync.dma_start(
                    out=upc[img, mch * 128 * 81:(mch + 1) * 128 * 81]
                    .rearrange("(p f) -> p f", f=81),
                    in_=pc_sb[:, (mch * BC + img) * 81:(mch * BC + img + 1) * 81])

        # ---- squash -> u_loc ----
        u_sb = ca.tile([128, BC * 21 * 8], F32, tag="usb")
        nc.vector.memset(u_sb[:], 0.0)
        for img in range(BC):
            nc.sync.dma_start(
                out=u_sb[:, img * 168:img * 168 + 160]
                .rearrange("p (c k) -> p c k", k=8),
                in_=_dap(upc, img * 20736, [[8, 128], [1024, 20], [1, 8]]))
            nc.sync.dma_start(
                out=u_sb[:32, img * 168 + 160:img * 168 + 168],
                in_=_dap(upc, img * 20736 + 20 * 1024, [[8, 32], [1, 8]]))
        n2 = ca.tile([128, BC * 21], F32, tag="sqn2")
        t1 = ca.tile([128, BC * 21], F32, tag="sqt1")
        r1 = ca.tile([128, BC * 21], F32, tag="sqr1")
        sq = ca.tile([128, BC * 168], F32, tag="sqsq")
        nc.scalar.activation(sq[:], u_sb[:], AF.Square)
        nc.vector.tensor_reduce(n2[:], sq[:].rearrange("p (c k) -> p c k", k=8),
                                AX.X, OP.add)
        nc.scalar.add(t1[:], n2[:], 1.0)
        nc.vector.reciprocal(t1[:], t1[:])
        nc.vector.tensor_scalar(t1[:], t1[:], -1.0, 1.0, OP.mult, OP.add)
        nc.vector.tensor_scalar_add(r1[:], n2[:], 1e-8)
        nc.scalar.activation(r1[:], r1[:], AF.Sqrt)
        nc.vector.reciprocal(r1[:], r1[:])
        nc.vector.tensor_mul(t1[:], t1[:], r1[:])
        nc.vector.tensor_mul(
            u_sb[:].rearrange("p (c k) -> p c k", k=8),
            u_sb[:].rearrange("p (c k) -> p c k", k=8),
            t1[:].rearrange("p (c one) -> p c one", one=1)
            .broadcast_to((128, BC * 21, 8)))
        u_bf = ca.tile([128, BC * 168], BF16, tag="ubf")
        nc.vector.tensor_copy(out=u_bf[:], in_=u_sb[:])
        for img in range(BC):
            nc.sync.dma_start(
                out=_dap(u_loc, img * 21504, [[8, 128], [1024, 21], [1, 8]]),
                in_=u_bf[:, img * 168:(img + 1) * 168]
                .rearrange("p (c k) -> p c k", k=8))

    # batch-shard -> i-shard via AllToAll
    nc.sync.dma_start(
        out=_dap(u_a2a, 0, [[5376, NCORES], [2688, BC], [1, 2688]]),
        in_=_dap(u_loc, 0, [[2688, NCORES], [21504, BC], [1, 2688]]))
    nc.gpsimd.collective_compute("AllToAll", OP.bypass, replica_groups=RG,
                                 ins=[u_a2a.opt()], outs=[u_mine.opt()])
    # u_mine as flat [16, 2688]: b-major blocks (core j's 2 images in order)

    # =========================================================
    # Phase B: routing (x_hat computed once in pass 0, cached bf16 in SBUF)
    # =========================================================
    with (
        tc.tile_pool(name="rt", bufs=1) as rt,
        tc.tile_pool(name="rtv", bufs=1) as rtv,
        tc.tile_pool(name="rcx", bufs=2) as rcx,
        tc.tile_pool(name="rex", bufs=2) as rex,
        tc.tile_pool(name="rsm", bufs=3) as rsm,
        tc.tile_pool(name="rsq", bufs=1) as rsq,
    ):
        rubd_cm = tc.tile_pool(name="rubd", bufs=1)
        rubd = rubd_cm.__enter__()
        rws_cm = tc.tile_pool(name="rws", bufs=2)
        rws = rws_cm.__enter__()
        ubd_sb = rubd.tile([64, G * 128], BF16, tag="ubd")
        with tc.tile_pool(name="rub", bufs=1) as rub:
            ubd_T = rub.tile([128, G * 64], BF16, tag="ubdT")
            nc.vector.memset(ubd_T[:], 0.0)
            for j in range(8):
                # diag block j as transposed: partitions 16j+b, cols (g, 8j+k)
                nc.sync.dma_start(
                    out=_pv(ubd_T, 16 * j, 8 * j, [["P", 16], [64, G], [1, 8]]),
                    in_=_dap(u_mine, j * 8, [[2688, 16], [64, G], [1, 8]]))
            ident = rub.tile([128, 128], BF16, tag="ident")
            from concourse.masks import make_identity
            make_identity(nc, ident[:])
            with tc.tile_pool(name="rtp", bufs=2, space="PSUM") as rtp:
                for g0 in range(0, G, 4):
                    gn = min(4, G - g0)
                    tp = rtp.tile([64, 512], BF16, tag="ptp")
                    for t in range(gn):
                        nc.tensor.transpose(
                            tp[:, t * 128:(t + 1) * 128],
                            ubd_T[:, (g0 + t) * 64:(g0 + t + 1) * 64],
                            ident[:])
                    nc.scalar.copy(ubd_sb[:, g0 * 128:(g0 + gn) * 128],
                                   tp[:, :gn * 128])
        # x_hat cache: [128, G*OD] bf16 (~134 KiB/partition)
        xh_sb = rt.tile([128, G * OD], BF16, tag="xh")
        rpx_cm = tc.tile_pool(name="rpx", bufs=2, space="PSUM")
        rpv_cm = tc.tile_pool(name="rpv", bufs=1, space="PSUM")
        rpx = rpx_cm.__enter__()
        rpv = rpv_cm.__enter__()
        smat_sb = rt.tile([128, 16], BF16, tag="smt")
        nc.sync.dma_start(out=smat_sb[:], in_=smat.ap())
        b_sb = rt.tile([128, G * O], BF16, tag="blog")
        vrep_sb = rt.tile([128, OD], BF16, tag="vrep")
        v_sb = rt.tile([16, OD], BF16, tag="vsb")
        vsum_sb = rt.tile([16, OD], BF16, tag="vsum")

        def squash16(src, dst):
            qn2 = rsq.tile([16, O], BF16, tag="q16a")
            qt = rsq.tile([16, O], BF16, tag="q16b")
            qr = rsq.tile([16, O], BF16, tag="q16c")
            qs = rsq.tile([16, OD], BF16, tag="vps")  # reuse vps slot
            with nc.allow_low_precision(reason="squash over 16 dims, bf16 ok"):
                nc.scalar.activation(qs[:], src, AF.Square)
                nc.vector.tensor_reduce(qn2[:],
                                        qs[:].rearrange("p (o d) -> p o d", d=D),
                                        AX.X, OP.add)
                nc.scalar.add(qt[:], qn2[:], 1.0)
                nc.vector.reciprocal(qt[:], qt[:])
                nc.vector.tensor_scalar(qt[:], qt[:], -1.0, 1.0, OP.mult, OP.add)
                nc.vector.tensor_scalar_add(qr[:], qn2[:], 1e-8)
                nc.scalar.activation(qr[:], qr[:], AF.Sqrt)
                nc.vector.reciprocal(qr[:], qr[:])
                nc.vector.tensor_mul(qt[:], qt[:], qr[:])
                nc.vector.tensor_mul(
                    dst.rearrange("p (o d) -> p o d", d=D),
                    src.rearrange("p (o d) -> p o d", d=D),
                    qt[:].rearrange("p (o one) -> p o one", one=1)
                    .broadcast_to((16, O, D)))

        def vrep_fill():
            for j in range(8):
                nc.sync.dma_start(out=vrep_sb[j * 16:(j + 1) * 16, :],
                                  in_=v_sb[:])

        # ---- pass 0: compute x_hat (cache bf16), v0 = squash(mean_i x_hat) ----
        HALVES = [(0, 816), (816, 1632)]
        pvp = rpv.tile([16, 2048], F32, tag="pvp")
        for g in range(G):
            wt = rws.tile([64, OD], BF16, tag="wt")
            nc.sync.dma_start(out=wt[:], in_=wrg[g, :, :])
            lhs = ubd_sb[:, g * 128:(g + 1) * 128]
            for h, (o0, o1) in enumerate(HALVES):
                X = rpx.tile([128, 1024], F32, tag="px")
                for (c0, c1) in ((o0, o0 + 512), (o0 + 512, o1)):
                    nc.tensor.matmul(X[:, c0 - o0:c1 - o0], lhs, wt[:, c0:c1],
                                     start=True, stop=True)
                xh_g = xh_sb[:, g * OD + o0:g * OD + o1]
                # split the PSUM->SBUF bf16 cast between scalar and vector
                if (2 * g + h) % 2 == 0:
                    nc.scalar.copy(xh_g, X[:, 0:o1 - o0])
                else:
                    nc.vector.tensor_copy(out=xh_g, in_=X[:, 0:o1 - o0])
        # second sweep: tight PE accumulation of s0 over cached x_hat
        for g in range(G):
            for (c0, c1) in _chunks512(OD):
                nc.tensor.matmul(pvp[:, c0:c1], smat_sb[:],
                                 xh_sb[:, g * OD + c0:g * OD + c1],
                                 start=(g == 0), stop=(g == G - 1),
                                 skip_group_check=True)
        rws_cm.__exit__(None, None, None)
        rubd_cm.__exit__(None, None, None)
        vps = rsq.tile([16, OD], BF16, tag="vps")
        nc.scalar.mul(vps[:], pvp[:, 0:OD], 1.0 / O)
        nc.sync.dma_start(out=v_in[0], in_=vps[:])
        nc.gpsimd.collective_compute("AllReduce", OP.add, replica_groups=RG,
                                     ins=[v_in[0].opt()], outs=[v_out[0].opt()])
        nc.sync.dma_start(out=vsum_sb[:], in_=v_out[0])
        squash16(vsum_sb[:], v_sb[:])
        vrep_fill()

        # ---- passes 1,2 (x_hat from SBUF cache; 3-group batched softmax) ----
        GB = 2
        for it in (1, 2):
            pvp = rpv.tile([16, 2048], F32, tag="pvp")
            for g0 in range(0, G, GB):
                xh3 = xh_sb[:, g0 * OD:(g0 + GB) * OD]
                tv3 = rtv.tile([128, GB * OD], BF16, tag="tv")
                vrep_b = vrep_sb[:].rearrange("p (one f) -> p one f", one=1) \
                    .broadcast_to((128, GB, OD))
                nc.vector.tensor_mul(
                    tv3[:].rearrange("p (t f) -> p t f", t=GB),
                    xh3.rearrange("p (t f) -> p t f", t=GB), vrep_b)
                bl3 = b_sb[:, g0 * O:(g0 + GB) * O]
                with nc.allow_low_precision(reason="b-logit d-reduce, 16 terms"):
                    # tree-sum over d=16 via contiguous bf16 adds (2x/4x DVE)
                    t8 = rtv.tile([128, GB * O * 8], BF16, tag="t8")
                    nc.vector.tensor_add(
                        t8[:].rearrange("p (t o z) -> p t o z", t=GB, z=8),
                        _pv(tv3, 0, 0, [[OD, GB], [D, O], [1, 8]]),
                        _pv(tv3, 0, 8, [[OD, GB], [D, O], [1, 8]]))
                    t4 = rtv.tile([128, GB * O * 4], BF16, tag="t4")
                    nc.vector.tensor_add(
                        t4[:].rearrange("p (t o z) -> p t o z", t=GB, z=4),
                        _pv(t8, 0, 0, [[O * 8, GB], [8, O], [1, 4]]),
                        _pv(t8, 0, 4, [[O * 8, GB], [8, O], [1, 4]]))
                    t2 = rtv.tile([128, GB * O * 2], BF16, tag="t2")
                    nc.vector.tensor_add(
                        t2[:].rearrange("p (t o z) -> p t o z", t=GB, z=2),
                        _pv(t4, 0, 0, [[O * 4, GB], [4, O], [1, 2]]),
                        _pv(t4, 0, 2, [[O * 4, GB], [4, O], [1, 2]]))
                    if it == 1:
                        nc.vector.tensor_add(
                            bl3.rearrange("p (t o) -> p t o", t=GB),
                            _pv(t2, 0, 0, [[O * 2, GB], [2, O]]),
                            _pv(t2, 0, 1, [[O * 2, GB], [2, O]]))
                    else:
                        tr3 = rtv.tile([128, GB * O], BF16, tag="tr")
                        nc.vector.tensor_add(
                            tr3[:].rearrange("p (t o) -> p t o", t=GB),
                            _pv(t2, 0, 0, [[O * 2, GB], [2, O]]),
                            _pv(t2, 0, 1, [[O * 2, GB], [2, O]]))
                        nc.vector.tensor_add(bl3, bl3, tr3[:])
                # softmax over o per (p, g): |bl| << 1 so no max-subtraction
                e3 = rsm.tile([128, GB * O], BF16, tag="e")
                nc.scalar.activation(e3[:], bl3, AF.Exp)
                s3 = rsm.tile([128, GB], BF16, tag="s")
                with nc.allow_low_precision(reason="softmax denom, 102 terms"):
                    nc.vector.tensor_reduce(
                        s3[:], e3[:].rearrange("p (t o) -> p t o", t=GB),
                        AX.X, OP.add)
                rs3 = rsm.tile([128, GB], BF16, tag="rs")
                with nc.allow_low_precision(reason="softmax recip bf16"):
                    nc.vector.reciprocal(rs3[:], s3[:])
                es3 = rsm.tile([128, GB * O], BF16, tag="es")
                rs3b = rs3[:].rearrange("p (t one) -> p t one", one=1) \
                    .broadcast_to((128, GB, O))
                nc.gpsimd.tensor_mul(
                    es3[:].rearrange("p (t o) -> p t o", t=GB),
                    e3[:].rearrange("p (t o) -> p t o", t=GB), rs3b)
                for t in range(GB):
                    g = g0 + t
                    esx = rex.tile([128, OD], BF16, tag="esx")
                    esb = es3[:, t * O:(t + 1) * O] \
                        .rearrange("p (o one) -> p o one", one=1) \
                        .broadcast_to((128, O, D))
                    nc.scalar.copy(
                        esx[:].rearrange("p (o d) -> p o d", d=D), esb)
                    cx = rcx.tile([128, OD], BF16, tag="cx")
                    nc.vector.tensor_mul(
                        cx[:], xh_sb[:, g * OD:(g + 1) * OD], esx[:])
                    for (c0, c1) in _chunks512(OD):
                        nc.tensor.matmul(pvp[:, c0:c1], smat_sb[:],
                                         cx[:, c0:c1],
                                         start=(g == 0), stop=(g == G - 1),
                                         skip_group_check=True)
            vps = rsq.tile([16, OD], BF16, tag="vps")
            nc.scalar.copy(vps[:], pvp[:, 0:OD])
            nc.sync.dma_start(out=v_in[it], in_=vps[:])
            nc.gpsimd.collective_compute(
                "AllReduce", OP.add, replica_groups=RG,
                ins=[v_in[it].opt()], outs=[v_out[it].opt()])
            nc.sync.dma_start(out=vsum_sb[:], in_=v_out[it])
            squash16(vsum_sb[:], v_sb[:])
            if it == 1:
                vrep_fill()

        nc.sync.dma_start(out=v2d.rearrange("(p f) -> p f", f=OD),
                          in_=v_sb[:])
        rpv_cm.__exit__(None, None, None)
        rpx_cm.__exit__(None, None, None)

    # =========================================================
    # Phase C: caps conv + FC head
    # =========================================================
    with (
        tc.tile_pool(name="fcw", bufs=1) as fcw,
        tc.tile_pool(name="fcs", bufs=6) as fcs,
        tc.tile_pool(name="fca", bufs=1) as fca,
        tc.tile_pool(name="fps", bufs=2, space="PSUM") as fps,
    ):
        caps3 = fca.tile([3, B * OD], BF16, tag="caps3")
        for kh in range(3):
            ln = B * OD - kh * D
            nc.sync.dma_start(
                out=caps3[kh:kh + 1, 0:ln],
                in_=v2d[kh * D:kh * D + ln].rearrange("(one f) -> one f", one=1))
        w3t_sb = fcw.tile([3, 768], BF16, tag="w3t")
        nc.sync.dma_start(out=w3t_sb[:], in_=w3T.ap())
        b3_sb = fcw.tile([128, 2], F32, tag="b3s")
        nc.sync.dma_start(out=b3_sb[:].rearrange("c (m one) -> c m one", one=1),
                          in_=b3c.ap().rearrange("m c one -> c m one"))
        h3_sb = fca.tile([128, 2 * B * 350], BF16, tag="h3")
        for mch in range(2):
            for b in range(B):
                ps = fps.tile([128, 512], F32, tag="ps3")
                for kw in range(3):
                    rhs = _pv(caps3, 0, b * OD + kw,
                              [["P", 3], [32, 50], [2, 7]])
                    nc.tensor.matmul(
                        ps[:, :350],
                        w3t_sb[:, (kw * 2 + mch) * 128:(kw * 2 + mch + 1) * 128],
                        rhs, start=(kw == 0), stop=(kw == 2))
                nc.scalar.activation(
                    h3_sb[:, mch * B * 350 + b * 350:mch * B * 350 + (b + 1) * 350],
                    ps[:, :350], AF.Relu, bias=b3_sb[:, mch:mch + 1])
        p3_sb = fca.tile([128, 2 * B * 72], BF16, tag="p3")
        hx_sb = fca.tile([128, B * 150], BF16, tag="hx")
        for mch in range(2):
            # x-pool 7->3 (k3 s2), then y-pool 50->24 (k3 s2); on gpsimd
            def h3x(kx):
                return _pv(h3_sb, 0, mch * B * 350 + kx,
                           [[350, B], [7, 50], [2, 3]])
            hxv = hx_sb[:].rearrange("p (b y x) -> p b y x", b=B, x=3)
            nc.vector.tensor_max(hxv, h3x(0), h3x(1))
            nc.vector.tensor_max(hxv, hxv, h3x(2))
            def hxy(ky):
                return _pv(hx_sb, 0, ky * 3, [[150, B], [6, 24], [1, 3]])
            # pos-major output: col = (oy*3+ox)*16 + b
            dst = _pv(p3_sb, 0, mch * B * 72, [[1, B], [48, 24], [16, 3]])
            nc.vector.tensor_max(dst, hxy(0), hxy(1))
            nc.vector.tensor_max(dst, dst, hxy(2))
        for mch in range(2):
            nc.sync.dma_start(
                out=_dap(fD2, mch * 128 * 72 * B,
                         [[72 * B, 128], [B, 72], [1, B]]),
                in_=_pv(p3_sb, 0, mch * B * 72, [[16, 72], [1, B]]))
        f_sb = fca.tile([128, 144 * B], BF16, tag="fsb")
        for hh in range(2):
            nc.gpsimd.dma_start(
                out=f_sb[:, hh * 72 * B:(hh + 1) * 72 * B]
                .rearrange("p (t b) -> p t b", b=B),
                in_=_dap(fD2, hh * 72 * 128 * B,
                         [[B, 128], [128 * B, 72], [1, B]]))

        def fc_layer(lhs_sb, n_kt, wstream, n_out, fbias, relu, fout_dram,
                     idx):
            KT = 4
            psf = fps.tile([16, 512], F32, tag="psf")
            for k4 in range(n_kt // KT):
                fwt = fcs.tile([128, KT * n_out], BF16, tag=f"fwt{idx}")
                q = nc.sync if k4 % 2 == 0 else nc.scalar
                q.dma_start(out=fwt[:], in_=wstream[k4, :, :])
                for t in range(KT):
                    kt = k4 * KT + t
                    nc.tensor.matmul(psf[:, :n_out],
                                     lhs_sb[:, kt * B:(kt + 1) * B],
                                     fwt[:, t * n_out:(t + 1) * n_out],
                                     start=(kt == 0), stop=(kt == n_kt - 1))
            fb_sb = fcw.tile([16, n_out], F32, tag=f"fb{idx}")
            nc.sync.dma_start(out=fb_sb[:], in_=fbias.ap())
            res = fca.tile([16, n_out], F32, tag=f"fr{idx}")
            nc.vector.tensor_add(res[:], psf[:, :n_out], fb_sb[:])
            if relu:
                nc.scalar.activation(res[:], res[:], AF.Relu)
            if fout_dram is not None:
                nc.sync.dma_start(
                    out=_dap(fout_dram, 0, [[1, 16], [16, n_out]]),
                    in_=res[:])
            return res

        fc_layer(f_sb, 144, fw1T, 512, fb1r, True, f1loc, 1)
        nc.gpsimd.collective_compute("AllGather", OP.bypass, replica_groups=RG,
                                     ins=[f1loc.opt()], outs=[f1g.opt()])
        f2_sb = fca.tile([128, 32 * B], BF16, tag="f2sb")
        nc.gpsimd.dma_start(out=f2_sb[:].rearrange("p (t b) -> p t b", b=B),
                            in_=_dap(f1g, 0, [[B, 128], [128 * B, 32], [1, B]]))
        fc_layer(f2_sb, 32, fw2T, 512, fb2r, True, f2loc, 2)
        nc.gpsimd.collective_compute("AllGather", OP.bypass, replica_groups=RG,
                                     ins=[f2loc.opt()], outs=[f2g.opt()])
        f3_sb = fca.tile([128, 32 * B], BF16, tag="f3sb")
        nc.gpsimd.dma_start(out=f3_sb[:].rearrange("p (t b) -> p t b", b=B),
                            in_=_dap(f2g, 0, [[B, 128], [128 * B, 32], [1, B]]))
        res3 = fc_layer(f3_sb, 32, fw3T, 102, fb3r, False, None, 3)
        nc.sync.dma_start(out=out_t[:, :], in_=res3[:])


def _prep_inputs(inputs):
    import ml_dtypes
    BF = ml_dtypes.bfloat16
    x = np.ascontiguousarray(inputs["x"], dtype=np.float32)
    w1, b1 = inputs["w1"], inputs["b1"]
    w2, b2 = inputs["w2"], inputs["b2"]
    wp, bp = inputs["wp"], inputs["bp"]
    Wcap = inputs["Wcap"]
    w3, b3 = inputs["w3"], inputs["b3"]
    fw1, fb1 = inputs["fw1"], inputs["fb1"]
    fw2, fb2 = inputs["fw2"], inputs["fb2"]
    fw3, fb3 = inputs["fw3"], inputs["fb3"]

    s = x.strides
    xw = as_strided(x, shape=(B, 3, 11, 11, 51, 51),
                    strides=(s[0], s[1], s[2], s[3], 4 * s[2], 4 * s[3]))
    xcols = np.ascontiguousarray(xw).reshape(B, 363, 2601)

    w1r = np.asarray(w1).reshape(96, 363)
    w1p = np.zeros((96, 384), np.float32)
    w1p[:, :363] = w1r
    w1T = np.ascontiguousarray(
        w1p.T.reshape(3, 128, 96).transpose(1, 0, 2)).reshape(128, 288)
    w2T = np.ascontiguousarray(
        np.asarray(w2).transpose(1, 2, 3, 0)).reshape(96, 6400)
    wpT = np.ascontiguousarray(
        np.asarray(wp).transpose(1, 2, 3, 0).reshape(2, 128, 4, 4, 256)
        .transpose(1, 2, 3, 0, 4)).reshape(128, 8192)
    w3T = np.ascontiguousarray(
        np.asarray(w3).reshape(256, 9).T.reshape(3, 3, 256)).reshape(3, 768)
    # w3T[kh, kw*256 + oc]  -> but kernel slices (kw*2+mch)*128: same layout.

    Wp = np.zeros((O, IPAD, D, 8), np.float32)
    Wp[:, :ITOT] = np.asarray(Wcap)
    wrg_all = np.ascontiguousarray(
        Wp.reshape(O, NCORES, G, 8, D, 8).transpose(1, 2, 3, 5, 0, 4)
    ).reshape(NCORES, G, 64, OD)

    fw1 = np.asarray(fw1); fw2 = np.asarray(fw2); fw3 = np.asarray(fw3)
    # fwT4[k4][p][t*n_out+o] = w[o_local, (4*k4+t)*128 + p]
    fw1T_all = np.ascontiguousarray(
        fw1.reshape(NCORES, 512, 36, 4, 128).transpose(0, 2, 4, 3, 1)
    ).reshape(NCORES, 36, 128, 2048)
    fw2T_all = np.ascontiguousarray(
        fw2.reshape(NCORES, 512, 8, 4, 128).transpose(0, 2, 4, 3, 1)
    ).reshape(NCORES, 8, 128, 2048)
    fw3T = np.ascontiguousarray(
        fw3.T.reshape(8, 4, 128, 102).transpose(0, 2, 1, 3)
    ).reshape(8, 128, 408)

    bf_names = {"xcols", "w1T", "w2T", "wpT", "w3T", "wrg", "smat",
                "fw1T", "fw2T", "fw3T"}
    shared = dict(
        w1T=w1T, b1c=np.asarray(b1).reshape(96, 1),
        w2T=w2T, b2c=np.asarray(b2).reshape(2, 128, 1),
        wpT=wpT, bpc=np.asarray(bp).reshape(2, 128, 1),
        w3T=w3T, b3c=np.asarray(b3).reshape(2, 128, 1),
        smat=np.tile(np.eye(16, dtype=np.float32), (8, 1)),
        fw3T=fw3T,
        fb3r=np.tile(np.asarray(fb3).reshape(1, 102), (16, 1)))
    in_maps = []
    for r in range(NCORES):
        m = dict(shared)
        m["xcols"] = xcols[2 * r:2 * r + 2]
        m["wrg"] = wrg_all[r]
        m["fw1T"] = fw1T_all[r]
        m["fw2T"] = fw2T_all[r]
        m["fb1r"] = np.tile(np.asarray(fb1)[512 * r:512 * (r + 1)].reshape(1, 512),
                            (16, 1))
        m["fb2r"] = np.tile(np.asarray(fb2)[512 * r:512 * (r + 1)].reshape(1, 512),
                            (16, 1))
        cast = {}
        for k, v in m.items():
            if k in bf_names:
                cast[k] = np.ascontiguousarray(v, dtype=BF)
            else:
                cast[k] = np.ascontiguousarray(v, dtype=np.float32)
        in_maps.append(cast)
    return in_maps


def kernel(**inputs):
    if "nc" not in _CACHE:
        _CACHE["nc"] = build_program()
    in_maps = _prep_inputs(inputs)
    last_err = None
    for attempt in range(3):
        try:
            res = run_bass_kernel_spmd(_CACHE["nc"], in_maps,
                                       list(range(NCORES)))
            _CACHE["last_exec_ns"] = res.exec_time_ns
            return np.asarray(res.results[0]["out"], dtype=np.float32)
        except Exception as err:  # transient device-unrecoverable states
            last_err = err
            import time as _time
            _time.sleep(20 * (attempt + 1))
    raise last_err


if __name__ == "__main__":
    data = np.load("/tmp/inputs.npz")
    inputs = {k: data[k] for k in data.files}
    out = kernel(**inputs)
    exp = np.load("/tmp/expected.npy")
    rel = np.abs(out - exp).max() / np.abs(exp).max()
    print(f"Relative error: {rel:.3e}")
